# revision 20
# baseline (speedup 1.0000x reference)
"""Trainium2 Bass kernel for nn_ClassificationLayer (Gaussian pdf-sum classifier).

Math:
  mu/sd per dim from tiny [128,10] reference sets (host, exact).
  Per row i: s_n[i] = sum_d INV_SQRT_2PI/sd_d * exp(-0.5*((x[i,d]-mu_d)/sd_d)^2)
  (same for anomaly), then the batch recurrence p_k = (p_{k-1} + s_k)/128,
  output = [pn/(pn+pa), pa/(pn+pa)].

Device strategy (8 cores, data-parallel over N, exact 62500-row shards):
  - Host transposes each core's shard to [128 dims, R rows]; per-dim constants
    become per-partition scale/bias vectors.
  - The elementwise Gaussian is split across TWO engines so neither is the
    bottleneck (ScalarE alone would need ~105us, above the ~90us HBM floor):
      * ScalarE: one ACTIVATE per distribution per tile computes
        Derivative_Erf(scale*x + bias) = (2/sqrt(pi)) * exp(-((x-mu)/sd)^2/2),
        output in bf16.
      * VectorE: a custom fused DVE op (registered at import) computes
        Schraudolph exp bits: out_i16 = max(BETA - (a'x + b')^2, 0) converted
        to int16, which *is* the bf16 bit pattern of ~exp(-((x-mu)/sd)^2/2).
        One DVE instruction per distribution per tile, ~1 elem/cycle/lane.
        Max per-element error ~3% (sawtooth); uniform bias cancels in the
        output ratio, measured end-to-end ratio error ~2e-3 on DVE rows.
  - Reduction over dims (partitions) via TensorEngine matvec in bf16 (2x the
    fp32 column rate). The stationary operand is a 64-wide shifted window over
    a zero-padded bf16 weight buffer so chunk g's sums land in PSUM partition
    g%64 of bank g//64; separate weight columns for ACT-produced (DErf scale)
    vs DVE-produced chunks. Bank A drains to DRAM under remaining compute.
  - The scalar recurrence decays by 1/128 per step, so it is re-run exactly on
    the gathered per-row sums on host as a short causal convolution in float64.
"""

import numpy as np

N, DIM, S = 500000, 128, 10
INV_SQRT_2PI = 0.3989422804014327
NCORES = 8
CHUNK = 512                      # rows per matvec (PSUM bank free-dim)
R = N // NCORES                  # 62500 rows per core, exact (no padding)
NCHUNK = -(-R // CHUNK)          # 123 chunks; last chunk has 36 rows
LAST_W = R - (NCHUNK - 1) * CHUNK

# Tile plan: (n_chunks, n_dve_chunks). DVE takes the first n_dve chunks of
# each tile, ScalarE the rest. Small head tiles start both engines early;
# the 1-chunk tail (36 cols) drains fast. DVE share 56/123 ~ 45.5% balances
# measured DVE (1.18 ns/col-dist) against ScalarE (1.02 ns/col-dist).
TILE_PLAN = ([(2, 1), (4, 2), (8, 4)]
             + [(9, 4), (9, 4), (9, 5), (9, 4), (9, 4), (9, 4),
                (9, 5), (9, 4), (9, 4), (9, 4), (9, 4), (9, 4)]
             + [(1, 0)])
assert sum(t[0] for t in TILE_PLAN) == NCHUNK
TILES = []           # (chunk0, n_chunks, n_dve)
_g = 0
for _nc_, _nd in TILE_PLAN:
    TILES.append((_g, _nc_, _nd))
    _g += _nc_
MAX_WV = max(nd * CHUNK for _, nd in TILE_PLAN)
MAX_WS = max((nc_ - nd) * CHUNK for nc_, nd in TILE_PLAN)
MAX_W = max(nc_ * CHUNK for nc_, _ in TILE_PLAN)

# chunk index after which PSUM bank A (chunks 0-63) is complete
FLUSH_AFTER_TILE = None
_cum = 0
for _ti, (_, _nc_, _) in enumerate(TILES):
    _cum += _nc_
    if _cum >= 64 and FLUSH_AFTER_TILE is None:
        FLUSH_AFTER_TILE = _ti

# Schraudolph-in-bf16-bit-space constants.
LOG2E = 1.4426950408889634
K_DVE = float(np.sqrt(128.0 * LOG2E))       # folds 2^7*log2(e) into a', b'
C_CENTER = 1.5 - 1.0 / float(np.log(2.0))   # zero-mean sawtooth offset
BETA = float(128.0 * (127.0 - C_CENTER))

# weight-window bases in the [128, 1024] weight buffer (c vector at the base
# column, zeros elsewhere; window [base-r, base-r+64) puts c at PSUM row r)
ACT_N_BASE, ACT_A_BASE = 128, 384
DVE_N_BASE, DVE_A_BASE = 640, 896

_COMPILED = None
_DVE_OP = None
LAST_RESULTS = None  # BassKernelResults of the most recent device run


def _register_dve_op():
    """Register the fused Schraudolph-exp custom DVE op at runtime."""
    global _DVE_OP
    if _DVE_OP is not None:
        return _DVE_OP
    import concourse.dve_ops as dvo
    from concourse.dve_spec import Spec, Src0, C0, C1, C2, Zero, maxx, sq, lower
    from concourse.dve_spec import _has_src1
    from concourse.dve_uop import DveOpSpec

    name = "GAUSS_EXP_BITS_ANT"
    if name in dvo._SUB_OPCODE_FOR_NAME:
        _DVE_OP = next(op for op in dvo.OPS if op.name == name)
        return _DVE_OP
    t = Src0 * C0 + C1
    spec = Spec(
        body=maxx(C2 - sq(t), Zero),
        reference=lambda in0, in1, s0, s1, imm2: np.maximum(
            np.float32(imm2)
            - (in0 * s0 + s1).astype(np.float32) ** 2,
            np.float32(0.0),
        ),
    )
    row = dvo._CUSTOM_DVE_ROW_BASE + len(dvo.OPS)
    shas = {}
    for ver in ("v3", "v4"):
        try:
            uops = lower(spec, ver=ver)
            shas[ver] = DveOpSpec(
                name=name, opcode=row, uops=uops, rd1_en=_has_src1(spec)
            ).sha(ver)
        except Exception:
            pass
    op = dvo.DveOp(name, spec, subdim=False, uops_sha=shas)
    dvo.OPS.append(op)
    dvo._SUB_OPCODE_FOR_NAME[name] = row
    dvo.CUSTOM_DVE_SPECS[name] = spec
    _DVE_OP = op
    return op


def _build():
    import concourse.tile as tile
    from concourse import bacc, mybir

    dve_op = _register_dve_op()

    nc = bacc.Bacc("TRN2", target_bir_lowering=False, debug=False,
                   num_devices=NCORES)

    xT = nc.dram_tensor("xT", [DIM, R], mybir.dt.float32,
                        kind="ExternalInput").ap()
    # consts cols: 0 scale_n, 1 bias_n, 2 scale_a, 3 bias_a (ACT);
    #              4 a'_n, 5 b'_n, 6 a'_a, 7 b'_a (DVE, scaled by K_DVE)
    consts = nc.dram_tensor("consts", [DIM, 8], mybir.dt.float32,
                            kind="ExternalInput").ap()
    wmat = nc.dram_tensor("wmat", [DIM, 1024], mybir.dt.bfloat16,
                          kind="ExternalInput").ap()
    sn_out = nc.dram_tensor("sn_out", [128, CHUNK], mybir.dt.float32,
                            kind="ExternalOutput").ap()
    sa_out = nc.dram_tensor("sa_out", [128, CHUNK], mybir.dt.float32,
                            kind="ExternalOutput").ap()

    DErf = mybir.ActivationFunctionType.Derivative_Erf
    bf16 = mybir.dt.bfloat16

    with tile.TileContext(nc) as tc:
        with tc.tile_pool(name="cpool", bufs=1) as cpool, \
             tc.tile_pool(name="xpool", bufs=6) as xpool, \
             tc.tile_pool(name="evpool", bufs=3) as evpool, \
             tc.tile_pool(name="espool", bufs=3) as espool, \
             tc.tile_pool(name="pspool", bufs=1, space="PSUM") as pspool:

            # consts first: tiny transfer that gates the table-load dummy
            # and the DVE scalar operands
            consts_t = cpool.tile([DIM, 8], mybir.dt.float32)
            nc.sync.dma_start(consts_t[:], consts[:, :])
            # prefetch the first x tiles so data is in flight while the
            # activation table loads
            x_pre = {}
            for ti in (0, 1):
                off, nch, _ = TILES[ti]
                w = min(nch * CHUNK, R - off * CHUNK)
                x_t = xpool.tile([DIM, w], mybir.dt.float32, tag="x",
                                 padded_shape=[DIM, MAX_W],
                                 name=f"x_pre{ti}")
                nc.sync.dma_start(x_t[:], xT[:, off * CHUNK:off * CHUNK + w])
                x_pre[ti] = x_t
            # Dummy activation: triggers the erf_derivative table load while
            # the first x tiles are still in flight. memset instead of a
            # consts read so the table load has no DMA dependency at all.
            warm_t = cpool.tile([DIM, 1], mybir.dt.float32)
            nc.vector.memset(warm_t[:], 0.0)
            nc.scalar.activation(warm_t[:], warm_t[:], DErf,
                                 bias=0.0, scale=1.0)
            # weights via SWDGE so the Sync HWDGE queue is x-tiles only
            w_t = cpool.tile([DIM, 1024], bf16)
            nc.gpsimd.dma_start(w_t[:], wmat[:, :])

            # per dist: bank A = chunks 0-63, bank B = chunks 64-122
            sn_psA = pspool.tile([64, CHUNK], mybir.dt.float32)
            sn_psB = pspool.tile([64, CHUNK], mybir.dt.float32)
            sa_psA = pspool.tile([64, CHUNK], mybir.dt.float32)
            sa_psB = pspool.tile([64, CHUNK], mybir.dt.float32)

            sn_sbA = cpool.tile([64, CHUNK], mybir.dt.float32)
            sa_sbA = cpool.tile([64, CHUNK], mybir.dt.float32)

            # last-emitted chunk per bank (PE program order: ACT chunks then
            # DVE chunks within each tile) — carries the stop flag
            emit_seq = []
            for g0, nch, ndv in TILES:
                for c in range(nch):
                    emit_seq.append(g0 + c)
            last_a = [g for g in emit_seq if g < 64][-1]
            last_b = [g for g in emit_seq if g >= 64][-1]
            mm_last = {(dd, gg) for dd in (0, 1) for gg in (last_a, last_b)}
            mm_started = {}

            for ti, (g0, nch, ndv) in enumerate(TILES):
                off = g0 * CHUNK
                w = min(nch * CHUNK, R - off)
                wv = ndv * CHUNK
                ws = w - wv
                if ti in x_pre:
                    x_t = x_pre[ti]
                else:
                    x_t = xpool.tile([DIM, w], mybir.dt.float32, tag="x",
                                     padded_shape=[DIM, MAX_W])
                    nc.sync.dma_start(x_t[:], xT[:, off:off + w])
                # Two waves per tile: produce n-dist results, run n matmuls
                # while the a-dist results are being produced, then a matmuls.
                # DVE chunks first within each wave: the PE starts as soon as
                # evn lands, and eva/esa arrive while the n-wave runs.
                # Emission order = PE execution order, so the start flag goes
                # on the first-emitted matmul per (dist, bank) — that matmul
                # initializes the whole bank.
                order = list(range(nch))
                ev_t = {}
                es_t = {}
                for dist in (0, 1):
                    if wv:
                        ev = evpool.tile([DIM, wv], bf16,
                                         tag="ev" + "na"[dist],
                                         padded_shape=[DIM, MAX_WV])
                        nc.vector._custom_dve(
                            dve_op, out=ev[:].bitcast(mybir.dt.int16),
                            in0=x_t[:, 0:wv],
                            s0=consts_t[:, 4 + 2 * dist:5 + 2 * dist],
                            s1=consts_t[:, 5 + 2 * dist:6 + 2 * dist],
                            imm2=BETA)
                        ev_t[dist] = ev
                    if ws:
                        es = espool.tile([DIM, ws], bf16,
                                         tag="es" + "na"[dist],
                                         padded_shape=[DIM, MAX_WS])
                        nc.scalar.activation(es[:], x_t[:, wv:w], DErf,
                                             bias=consts_t[:, 1 + 2 * dist:
                                                           2 + 2 * dist],
                                             scale=consts_t[:, 2 * dist:
                                                            1 + 2 * dist])
                        es_t[dist] = es
                    ps_banks = (sn_psA, sn_psB) if dist == 0 else \
                               (sa_psA, sa_psB)
                    for c in order:
                        g = g0 + c
                        r = g % 64
                        cw = min(CHUNK, w - c * CHUNK)
                        if c < ndv:
                            rhs = ev_t[dist][:, c * CHUNK:c * CHUNK + cw]
                            base = DVE_N_BASE if dist == 0 else DVE_A_BASE
                        else:
                            o2 = (c - ndv) * CHUNK
                            rhs = es_t[dist][:, o2:o2 + cw]
                            base = ACT_N_BASE if dist == 0 else ACT_A_BASE
                        bank = 0 if g < 64 else 1
                        ps = ps_banks[bank]
                        key = (dist, bank)
                        first = key not in mm_started
                        mm_started[key] = True
                        last = (dist, g) in mm_last
                        nc.tensor.matmul(ps[:, 0:cw],
                                         w_t[:, base - r:base - r + 64],
                                         rhs, start=first, stop=last,
                                         skip_group_check=True)
                if ti == FLUSH_AFTER_TILE + 1:
                    # bank A complete: drain it under the remaining compute
                    # (one tile late so DVE doesn't stall on the matmuls);
                    # out-DMAs on the idle SWDGE queue, not the x-tile queue
                    nc.vector.tensor_copy(sn_sbA[:], sn_psA[:])
                    nc.vector.tensor_copy(sa_sbA[:], sa_psA[:])
                    nc.gpsimd.dma_start(sn_out[0:64, :], sn_sbA[:])
                    nc.gpsimd.dma_start(sa_out[0:64, :], sa_sbA[:])

            # drain bank B on two different engines so the copies overlap;
            # the two out-DMAs dispatch from two different HWDGE rings
            sn_sbB = cpool.tile([64, CHUNK], mybir.dt.float32)
            nc.vector.tensor_copy(sn_sbB[:], sn_psB[:])
            sa_sbB = cpool.tile([64, CHUNK], mybir.dt.float32)
            nc.scalar.copy(sa_sbB[:], sa_psB[:])
            nc.sync.dma_start(sn_out[64:128, :], sn_sbB[:])
            nc.scalar.dma_start(sa_out[64:128, :], sa_sbB[:])

    nc.compile()
    return nc


def _get_compiled():
    global _COMPILED
    if _COMPILED is None:
        _COMPILED = _build()
    return _COMPILED


def kernel(encoded, normal_dist, anomaly_dist):
    global LAST_RESULTS
    import ml_dtypes
    from concourse.bass_utils import run_bass_kernel_spmd

    x = np.ascontiguousarray(np.asarray(encoded, dtype=np.float32))
    nd = np.asarray(normal_dist, dtype=np.float64)
    ad = np.asarray(anomaly_dist, dtype=np.float64)

    # per-dim stats (torch defaults: unbiased std)
    mu_n = nd.mean(axis=1)
    sd_n = nd.std(axis=1, ddof=1)
    mu_a = ad.mean(axis=1)
    sd_a = ad.std(axis=1, ddof=1)
    isd_n, isd_a = 1.0 / sd_n, 1.0 / sd_a

    inv_sqrt2 = 1.0 / np.sqrt(2.0)
    scale_n = isd_n * inv_sqrt2
    bias_n = -mu_n * isd_n * inv_sqrt2
    scale_a = isd_a * inv_sqrt2
    bias_a = -mu_a * isd_a * inv_sqrt2
    consts = np.stack([
        scale_n, bias_n, scale_a, bias_a,
        K_DVE * scale_n, K_DVE * bias_n,
        K_DVE * scale_a, K_DVE * bias_a,
    ], axis=1).astype(np.float32)     # [128, 8]

    half_sqrt_pi = 0.5 * np.sqrt(np.pi)
    wmat = np.zeros((DIM, 1024), dtype=ml_dtypes.bfloat16)
    wmat[:, ACT_N_BASE] = (INV_SQRT_2PI * isd_n * half_sqrt_pi).astype(
        ml_dtypes.bfloat16)
    wmat[:, ACT_A_BASE] = (INV_SQRT_2PI * isd_a * half_sqrt_pi).astype(
        ml_dtypes.bfloat16)
    wmat[:, DVE_N_BASE] = (INV_SQRT_2PI * isd_n).astype(ml_dtypes.bfloat16)
    wmat[:, DVE_A_BASE] = (INV_SQRT_2PI * isd_a).astype(ml_dtypes.bfloat16)

    in_maps = []
    for i in range(NCORES):
        lo = i * R
        shard_T = np.ascontiguousarray(x[lo:lo + R].T)   # [128, R]
        in_maps.append({"xT": shard_T, "consts": consts, "wmat": wmat})

    nc = _get_compiled()
    try:
        res = run_bass_kernel_spmd(nc, in_maps, core_ids=list(range(NCORES)))
    except Exception:
        # one retry: the NRT occasionally reports a transient
        # NRT_EXEC_UNIT_UNRECOVERABLE on an otherwise-healthy device
        res = run_bass_kernel_spmd(nc, in_maps, core_ids=list(range(NCORES)))
    LAST_RESULTS = res

    s_n = np.empty(N, dtype=np.float64)
    s_a = np.empty(N, dtype=np.float64)
    for i in range(NCORES):
        lo = i * R
        s_n[lo:lo + R] = res.results[i]["sn_out"].reshape(-1)[:R]
        s_a[lo:lo + R] = res.results[i]["sa_out"].reshape(-1)[:R]

    # exact recurrence p_k = (p_{k-1} + s_k)/dim as truncated causal
    # convolution: p_k = sum_j (1/dim)^(j+1) s_{k-j}; (1/128)^14 ~ 3e-30.
    a = 1.0 / DIM
    pn = np.zeros(N, dtype=np.float64)
    pa = np.zeros(N, dtype=np.float64)
    wgt = a
    for j in range(14):
        if j == 0:
            pn += wgt * s_n
            pa += wgt * s_a
        else:
            pn[j:] += wgt * s_n[:-j]
            pa[j:] += wgt * s_a[:-j]
        wgt *= a
    total = pn + pa
    out = np.empty((N, 2), dtype=np.float32)
    out[:, 0] = (pn / total).astype(np.float32)
    out[:, 1] = (pa / total).astype(np.float32)
    return out
